# revision 1
# baseline (speedup 1.0000x reference)
"""Trainium2 Bass kernel for the 4-layer GCN + mesh-unpool network
(nn_Net_7060926234635), distributed across 8 NeuronCores.

Strategy (dst-sharded graph parallelism, one NEFF for all 8 cores):
  * Host side (pure index/permutation preprocessing + rsqrt-degree
    normalization constants): per-core padded-CSR slot grids over
    in-degree-sorted node shards, composed gather indices, AllToAll
    fetch plans for the unpool gathers, and final-output reassembly maps.
  * Device side per core: indirect-DMA row gathers + DVE segmented
    reduction for message aggregation, PE matmuls for the feature
    transforms, ACT relu/scales; AllGather for the message tables,
    AllToAll for the unpool row exchange.

The kernel accepts the FULL (unsharded) inputs exactly as produced by
setup_inputs() and returns the full [1600000, 3] float32 output.
Biases b1..b4 are zeros per the problem spec; asserted below.
"""
import sys
sys.path.insert(0, "/opt/trn_rl_repo")

import numpy as np

import numpy as np

NC = 8
P = 128


def pad_to(x, m):
    return (x + m - 1) // m * m


# ----------------------------------------------------------------------------
# host-side planning
# ----------------------------------------------------------------------------

def make_dis(edge_index, n):
    deg = np.bincount(edge_index[1], minlength=n).astype(np.float64) + 1.0
    return (1.0 / np.sqrt(deg)).astype(np.float32)


def plan_agg(dst_old, slot_row, n, self_row, G, slot_weight=None,
             self_weight=None, zero_row=0, col_budget=128):
    """Padded-CSR aggregation plan, program-unified across cores.

    Returns dict:
      groups : list of (Gg, Kt) per supertile group (same for all cores)
      idx[c] : int32 [P, Ctot] slot grids (col j of group g at base_g +
               gsub*Kt + k)
      w[c]   : fp32 [P, Ctot] or None
      dis[c] : fp32 [P, ntiles] per-tile dst scale columns (new-local order)
      sigma  : old id -> global new id (c * shardP + local_new)
      shardP : padded shard rows
    """
    shard = n // NC
    shardP = pad_to(shard, P)
    ntiles = shardP // P
    c_of = dst_old // shard
    has_w = slot_weight is not None

    percore = []
    for c in range(NC):
        m = c_of == c
        dl = dst_old[m] - c * shard
        deg = np.bincount(dl, minlength=shard)
        perm = np.argsort(-deg, kind="stable")
        inv = np.empty_like(perm); inv[perm] = np.arange(shard)
        percore.append((m, dl, deg, perm, inv))

    # unified group structure (Gg*Kt capped by col_budget for SBUF sizing)
    def span_kt(lo_t, Gg):
        lo, hi = lo_t * P, min((lo_t + Gg) * P, shard)
        kt = 0
        for c in range(NC):
            deg_new = percore[c][2][percore[c][3]]
            if hi > lo:
                kt = max(kt, int(deg_new[lo:hi].max()))
        return kt + 1
    groups = []
    t = 0
    while t < ntiles:
        Gg = min(G, ntiles - t)
        kt = span_kt(t, Gg)
        while Gg > 1 and Gg * kt > col_budget:
            Gg = max(1, min(Gg - 1, col_budget // kt))
            kt = span_kt(t, Gg)
        groups.append((Gg, kt))
        t += Gg

    # column base per tile
    colbase = np.zeros(ntiles + 1, np.int64)
    kts_tile = []
    for (Gg, kt) in groups:
        kts_tile += [kt] * Gg
    for t in range(ntiles):
        colbase[t + 1] = colbase[t] + kts_tile[t]
    Ctot = int(colbase[-1])

    out = {"groups": groups, "idx": [], "w": [], "dis": [], "perm": [],
           "shardP": shardP, "Ctot": Ctot}
    sigma = np.empty(n, np.int64)
    for c in range(NC):
        m, dl, deg, perm, inv = percore[c]
        sigma[c * shard:(c + 1) * shard] = c * shardP + inv
        sr = slot_row[m]
        sw = slot_weight[m] if has_w else None
        nd = inv[dl]
        order = np.argsort(nd, kind="stable")
        nd_s = nd[order]; sr_s = sr[order]
        deg_new = deg[perm]
        starts = np.zeros(shard + 1, np.int64)
        np.cumsum(deg_new, out=starts[1:])
        idx = np.full((P, Ctot), zero_row, np.int64)
        w = np.zeros((P, Ctot), np.float32) if has_w else None
        r = np.arange(len(nd_s)) - starts[nd_s]
        pp_ = nd_s % P
        tt_ = nd_s // P
        cols = colbase[tt_] + r
        idx[pp_, cols] = sr_s
        if has_w:
            w[pp_, cols] = sw[order]
        v = np.arange(shard)
        scols = colbase[v // P] + deg_new[v]
        idx[v % P, scols] = self_row[perm + c * shard]
        if has_w:
            w[v % P, scols] = self_weight[perm + c * shard]
        out["idx"].append(idx.astype(np.int32))
        out["w"].append(w)
        out["perm"].append(perm)
    out["sigma"] = sigma
    return out


def plan_fetch(u, shardP_coarse, sigma_coarse, n_fine_shard):
    """Owner-side gather + exchange plan for x[i] = A[u[i]].

    Per owner o, per requester c: list of requested owner-local rows,
    ordered by fine id; blocks padded to common B.
    Returns send grids [P, NC*B/P] per owner (col-major: flat j=col*128+p),
    B, and rho[c][i_local] = row position in c's recv buffer [NC*B].
    """
    n_fine = len(u)
    gid = sigma_coarse[u]
    owner = gid // shardP_coarse
    loc = gid % shardP_coarse
    req = np.arange(n_fine) // n_fine_shard
    counts = np.zeros((NC, NC), np.int64)
    lists = [[None] * NC for _ in range(NC)]
    for o in range(NC):
        mo = owner == o
        for c in range(NC):
            m = mo & (req == c)
            ii = np.nonzero(m)[0]
            lists[o][c] = (ii, loc[m])
            counts[o, c] = len(ii)
    B = int(pad_to(counts.max(), P))
    send_idx = []
    for o in range(NC):
        rows = np.zeros((NC, B), np.int64)
        for c in range(NC):
            ii, ll = lists[o][c]
            rows[c, :len(ll)] = ll
        flat = rows.reshape(NC * B)
        send_idx.append(flat.reshape((NC * B) // P, P).T.astype(np.int32).copy())
    rho = []
    for c in range(NC):
        pos = np.empty(n_fine_shard, np.int64)
        for o in range(NC):
            ii, _ = lists[o][c]
            pos[ii - c * n_fine_shard] = o * B + np.arange(len(ii))
        rho.append(pos)
    return send_idx, B, rho


def tile_cols(vec_percore, shardP):
    """[shard]-per-core vector -> [P, ntiles] column layout (pad 0)."""
    outs = []
    for v in vec_percore:
        a = np.zeros(shardP, np.float32)
        a[:len(v)] = v
        outs.append(a.reshape(shardP // P, P).T.copy())
    return outs


def prep(inputs, G1=16, G2=4, G3=8, G4=16, GB3=2, GB4=4, GS=8):
    """Full host plan. Returns (meta, in_maps, reassemble).

    meta drives the (single) Bass program; in_maps are per-core inputs;
    reassemble(outs) -> final [n4fine, 3] output.
    """
    x = np.asarray(inputs["x"], np.float32)
    W = [np.asarray(inputs[f"W{i}"], np.float32) for i in (1, 2, 3, 4)]
    ei = [np.asarray(inputs[f"edge_index{i}"]).astype(np.int64) for i in range(4)]
    u = [np.asarray(inputs[f"unpool{i}"]).astype(np.int64) for i in (1, 2, 3, 4)]
    n = [x.shape[0], len(u[0]), len(u[1]), len(u[2])]
    nout = len(u[3])
    dis = [make_dis(ei[l], n[l]) for l in range(4)]

    # L1: table = x0p (original order), weighted by dis1[src]
    L1 = plan_agg(ei[0][1], ei[0][0], n[0], self_row=np.arange(n[0]), G=G1,
                  slot_weight=dis[0][ei[0][0]], self_weight=dis[0])
    s1 = L1["sigma"]
    # L2: table = A1 (global new order), composed via u1, weighted by dis2[src]
    L2 = plan_agg(ei[1][1], s1[u[0][ei[1][0]]], n[1], self_row=s1[u[0]], G=G2,
                  slot_weight=dis[1][ei[1][0]], self_weight=dis[1])
    s2 = L2["sigma"]
    # C2: fetch A2 rows for x3
    sh3 = n[2] // NC
    f3_send, B3, rho3 = plan_fetch(u[1], L2["shardP"], s2, sh3)
    S3 = NC * B3 + P                     # g3 shard rows (+zero tile)
    g3row = np.empty(n[2], np.int64)
    for c in range(NC):
        g3row[c * sh3:(c + 1) * sh3] = c * S3 + rho3[c]
    L3 = plan_agg(ei[2][1], g3row[ei[2][0]], n[2], self_row=g3row, G=G3,
                  zero_row=NC * B3)
    s3 = L3["sigma"]
    sh4 = n[3] // NC
    f4_send, B4, rho4 = plan_fetch(u[2], L3["shardP"], s3, sh4)
    S4 = NC * B4 + P
    g4row = np.empty(n[3], np.int64)
    for c in range(NC):
        g4row[c * sh4:(c + 1) * sh4] = c * S4 + rho4[c]
    L4 = plan_agg(ei[3][1], g4row[ei[3][0]], n[3], self_row=g4row, G=G4,
                  zero_row=NC * B4)
    s4 = L4["sigma"]
    fin_send, Bf, rhoF = plan_fetch(u[3], L4["shardP"], s4, nout // NC)

    # per-layer dst dis in tile-column layout
    dis_dst = []
    for l, L in enumerate((L1, L2, L3, L4)):
        sh = n[l] // NC
        dis_dst.append(tile_cols(
            [dis[l][c * sh + L["perm"][c]] for c in range(NC)], L["shardP"]))
    # recv-order dis for g3/g4 build
    disg = []
    for (uu, rho, B, dd) in ((u[1], rho3, B3, dis[2]), (u[2], rho4, B4, dis[3])):
        shf = len(uu) // NC // 1
        shf = len(uu) // NC
        per = []
        for c in range(NC):
            a = np.zeros(NC * B, np.float32)
            a[rho[c]] = dd[c * shf:(c + 1) * shf]
            per.append(a.reshape((NC * B) // P, P).T.copy())
        disg.append(per)

    # weight padding for L1/L2 grids is already 0 => padded slots vanish.
    meta = dict(
        n=n, nout=nout,
        L1=dict(groups=L1["groups"], shardP=L1["shardP"], C=L1["Ctot"],
                fin=4, fout=32, weighted=True, matmul=True, relu=True),
        L2=dict(groups=L2["groups"], shardP=L2["shardP"], C=L2["Ctot"],
                fin=32, fout=64, weighted=True, matmul=True, relu=True),
        L3=dict(groups=L3["groups"], shardP=L3["shardP"], C=L3["Ctot"],
                fin=32, fout=32, weighted=False, matmul=False, relu=True),
        L4=dict(groups=L4["groups"], shardP=L4["shardP"], C=L4["Ctot"],
                fin=4, fout=4, weighted=False, matmul=False, relu=True),
        B3=B3, S3=S3, B4=B4, S4=S4, Bf=Bf,
        b3=dict(fin=64, fout=32, G=GB3),    # g3 build
        b4=dict(fin=32, fout=4, G=GB4),     # g4 build
        GS=GS,
        table1_rows=n[0],
        A1_rows=NC * L1["shardP"],
        g3_rows=NC * S3, g4_rows=NC * S4,
    )

    # ---- per-core input tensors ----
    x0p = np.zeros((n[0], 4), np.float32)
    x0p[:, :3] = x
    W1p = np.zeros((4, 32), np.float32); W1p[:3] = W[0]
    W4p = np.zeros((32, 4), np.float32); W4p[:, :3] = W[3]

    def blkdiag(Wm, G):
        fi, fo = Wm.shape
        B = np.zeros((G * fi, G * fo), np.float32)
        for g in range(G):
            B[g * fi:(g + 1) * fi, g * fo:(g + 1) * fo] = Wm
        return B

    ident = np.eye(P, dtype=np.float32)
    in_maps = []
    for c in range(NC):
        m = {
            "x0p": x0p, "ident": ident,
            "w1blk": blkdiag(W1p, G1), "w2blk": blkdiag(W[1], G2),
            "w3blk": blkdiag(W[2], GB3), "w4blk": blkdiag(W4p, GB4),
            "idx1": L1["idx"][c], "wg1": L1["w"][c], "dis1d": dis_dst[0][c],
            "idx2": L2["idx"][c], "wg2": L2["w"][c], "dis2d": dis_dst[1][c],
            "sidx3": f3_send[c], "dis3r": disg[0][c],
            "idx3": L3["idx"][c], "dis3d": dis_dst[2][c],
            "sidx4": f4_send[c], "dis4r": disg[1][c],
            "idx4": L4["idx"][c], "dis4d": dis_dst[3][c],
            "fidx": fin_send[c],
        }
        in_maps.append(m)

    def reassemble(outs):
        res = np.empty((nout, 3), np.float32)
        shf = nout // NC
        # requester c's recv would be concat_o of block[o][c]; but we do it on
        # host: owner o's outbuf rows (col-major grid order) are, for dest c,
        # rows [c*Bf:(c+1)*Bf]; position rho maps fine idx -> o*Bf + k.
        for c in range(NC):
            # gather rows for requester c from each owner's outbuf
            # rhoF[c][i] = o*Bf + k ; row in owner o's outbuf at c*Bf + k
            pos = rhoF[c]
            o = pos // Bf
            k = pos % Bf
            rows = np.empty((shf, 4), np.float32)
            for oo in range(NC):
                mm = o == oo
                rows[mm] = outs[oo]["outbuf"][c * Bf + k[mm]]
            res[c * shf:(c + 1) * shf] = rows[:, :3]
        return res

    return meta, in_maps, reassemble


# ----------------------------------------------------------------------------
# device kernel builder
# ----------------------------------------------------------------------------

def build_kernel(meta, stop_after=None, dbg_tensor=None):
    import concourse.bass as bass
    import concourse.mybir as mybir
    from concourse.bacc import Bacc
    from concourse.tile import TileContext
    from concourse.bass import IndirectOffsetOnAxis

    f32 = mybir.dt.float32
    i32 = mybir.dt.int32
    n = meta["n"]

    nc = Bacc("TRN2", target_bir_lowering=False, debug=False, num_devices=NC)

    # ---------------- inputs ----------------
    x0p = nc.dram_tensor("x0p", [n[0], 4], f32, kind="ExternalInput")
    ident_d = nc.dram_tensor("ident", [P, P], f32, kind="ExternalInput")
    w1blk_d = nc.dram_tensor("w1blk", [16 * 4, 16 * 32], f32, kind="ExternalInput")
    w2blk_d = nc.dram_tensor("w2blk", [4 * 32, 4 * 64], f32, kind="ExternalInput")
    w3blk_d = nc.dram_tensor("w3blk", [meta["b3"]["G"] * 64, meta["b3"]["G"] * 32], f32, kind="ExternalInput")
    w4blk_d = nc.dram_tensor("w4blk", [meta["b4"]["G"] * 32, meta["b4"]["G"] * 4], f32, kind="ExternalInput")

    def grid_in(name, C, dt=i32):
        return nc.dram_tensor(name, [P, C], dt, kind="ExternalInput")

    L1, L2, L3, L4 = meta["L1"], meta["L2"], meta["L3"], meta["L4"]
    idx1 = grid_in("idx1", L1["C"]); wg1 = grid_in("wg1", L1["C"], f32)
    dis1d = grid_in("dis1d", L1["shardP"] // P, f32)
    idx2 = grid_in("idx2", L2["C"]); wg2 = grid_in("wg2", L2["C"], f32)
    dis2d = grid_in("dis2d", L2["shardP"] // P, f32)
    sidx3 = grid_in("sidx3", (NC * meta["B3"]) // P)
    dis3r = grid_in("dis3r", (NC * meta["B3"]) // P, f32)
    idx3 = grid_in("idx3", L3["C"])
    dis3d = grid_in("dis3d", L3["shardP"] // P, f32)
    sidx4 = grid_in("sidx4", (NC * meta["B4"]) // P)
    dis4r = grid_in("dis4r", (NC * meta["B4"]) // P, f32)
    idx4 = grid_in("idx4", L4["C"])
    dis4d = grid_in("dis4d", L4["shardP"] // P, f32)
    fidx = grid_in("fidx", (NC * meta["Bf"]) // P)

    outbuf = nc.dram_tensor("outbuf", [NC * meta["Bf"], 4], f32, kind="ExternalOutput")

    rg = [list(range(NC))]

    dbg = None
    if stop_after is not None:
        dbg_rows, dbg_f = dbg_tensor
        dbg = nc.dram_tensor("dbg", [dbg_rows, dbg_f], f32, kind="ExternalOutput")

    with TileContext(nc) as tc:
        with (
            tc.tile_pool(name="dramp", bufs=1, space="DRAM") as dramp,
            tc.tile_pool(name="consts", bufs=1) as constp,
            tc.tile_pool(name="idxp", bufs=3) as idxp,
            tc.tile_pool(name="gath", bufs=2) as gathp,
            tc.tile_pool(name="work", bufs=3) as workp,
            tc.tile_pool(name="outp", bufs=3) as outp,
            tc.tile_pool(name="psumT", bufs=2, space="PSUM") as psumTp,
            tc.tile_pool(name="psumM", bufs=2, space="PSUM") as psumMp,
        ):
            # ------- persistent DRAM intermediates -------
            A1sh = dramp.tile([L1["shardP"], 32], f32)
            A1f = dramp.tile([NC * L1["shardP"], 32], f32, addr_space="Shared")
            A2sh = dramp.tile([L2["shardP"], 64], f32)
            send3 = dramp.tile([NC * meta["B3"], 64], f32)
            recv3 = dramp.tile([NC * meta["B3"], 64], f32)
            g3sh = dramp.tile([meta["S3"], 32], f32)
            g3f = dramp.tile([NC * meta["S3"], 32], f32, addr_space="Shared")
            A3sh = dramp.tile([L3["shardP"], 32], f32)
            send4 = dramp.tile([NC * meta["B4"], 32], f32)
            recv4 = dramp.tile([NC * meta["B4"], 32], f32)
            g4sh = dramp.tile([meta["S4"], 4], f32)
            g4f = dramp.tile([NC * meta["S4"], 4], f32, addr_space="Shared")
            A4sh = dramp.tile([L4["shardP"], 4], f32)

            # ------- constants to SBUF -------
            ident = constp.tile([P, P], f32)
            nc.sync.dma_start(out=ident[:], in_=ident_d[:, :])
            w1b = constp.tile([64, 16 * 32], f32)
            nc.sync.dma_start(out=w1b[:], in_=w1blk_d[:, :])
            w2b = constp.tile([128, 4 * 64], f32)
            nc.sync.dma_start(out=w2b[:], in_=w2blk_d[:, :])
            w3b = constp.tile([meta["b3"]["G"] * 64, meta["b3"]["G"] * 32], f32)
            nc.sync.dma_start(out=w3b[:], in_=w3blk_d[:, :])
            w4b = constp.tile([meta["b4"]["G"] * 32, meta["b4"]["G"] * 4], f32)
            nc.sync.dma_start(out=w4b[:], in_=w4blk_d[:, :])

            # =========== generic aggregation phase ===========
            def agg_phase(lm, idx_d, w_d, disd_d, table_ap, out_dram, wblk,
                          tag):
                fin, fout = lm["fin"], lm["fout"]
                t0 = 0          # running tile index
                col = 0
                for gi, (Gg, kt) in enumerate(lm["groups"]):
                    ncols = Gg * kt
                    idxt = idxp.tile([P, ncols], i32, tag=f"idx{tag}",
                                     name=f"idx{tag}_{gi}")
                    nc.sync.dma_start(out=idxt[:], in_=idx_d[:, col:col + ncols])
                    gt = gathp.tile([P, ncols * fin], f32, tag=f"g{tag}",
                                    name=f"g{tag}_{gi}")
                    for j in range(ncols):
                        nc.gpsimd.indirect_dma_start(
                            out=gt[:, j * fin:(j + 1) * fin],
                            out_offset=None,
                            in_=table_ap,
                            in_offset=IndirectOffsetOnAxis(
                                ap=idxt[:, j:j + 1], axis=0),
                        )
                    dcol = idxp.tile([P, Gg], f32, tag=f"d{tag}",
                                     name=f"d{tag}_{gi}")
                    nc.sync.dma_start(out=dcol[:], in_=disd_d[:, t0:t0 + Gg])
                    if lm["weighted"]:
                        wt = idxp.tile([P, ncols], f32, tag=f"w{tag}",
                                       name=f"w{tag}_{gi}")
                        nc.sync.dma_start(out=wt[:], in_=w_d[:, col:col + ncols])
                        nc.vector.tensor_tensor(
                            out=gt[:].rearrange("p (c f) -> p c f", f=fin),
                            in0=gt[:].rearrange("p (c f) -> p c f", f=fin),
                            in1=wt[:].rearrange("p c -> p c").to_broadcast(
                                [P, ncols, fin]),
                            op=mybir.AluOpType.mult)
                        red_in = gt
                    else:
                        red_in = gt
                    # reduce over k: view [P, Gg, fin, kt]
                    S = workp.tile([P, Gg * fin], f32, tag=f"S{tag}",
                                   name=f"S{tag}_{gi}")
                    nc.vector.tensor_reduce(
                        out=S[:].rearrange("p (g f) -> p g f", f=fin),
                        in_=red_in[:].rearrange("p (g k f) -> p g f k",
                                                g=Gg, k=kt),
                        axis=mybir.AxisListType.X, op=mybir.AluOpType.add)
                    if lm["matmul"]:
                        # scale by dis BEFORE matmul: S *= dis (per p,g)
                        nc.vector.tensor_tensor(
                            out=S[:].rearrange("p (g f) -> p g f", f=fin),
                            in0=S[:].rearrange("p (g f) -> p g f", f=fin),
                            in1=dcol[:].to_broadcast([P, Gg, fin]),
                            op=mybir.AluOpType.mult)
                        pT = psumTp.tile([Gg * fin, P], f32, tag="pT",
                                         name=f"pT{tag}_{gi}")
                        nc.tensor.transpose(out=pT[:], in_=S[:],
                                            identity=ident[:])
                        ST = workp.tile([Gg * fin, P], f32, tag=f"ST{tag}",
                                        name=f"ST{tag}_{gi}")
                        nc.scalar.copy(out=ST[:], in_=pT[:])
                        pM = psumMp.tile([P, Gg * fout], f32, tag="pM",
                                         name=f"pM{tag}_{gi}")
                        nc.tensor.matmul(out=pM[:], lhsT=ST[:],
                                         rhs=wblk[:Gg * fin, :Gg * fout],
                                         start=True, stop=True)
                        at = outp.tile([P, Gg * fout], f32, tag=f"A{tag}",
                                       name=f"A{tag}_{gi}")
                        nc.scalar.activation(
                            out=at[:], in_=pM[:],
                            func=mybir.ActivationFunctionType.Relu)
                    else:
                        # out = relu(dis * S)
                        nc.vector.tensor_tensor(
                            out=S[:].rearrange("p (g f) -> p g f", f=fin),
                            in0=S[:].rearrange("p (g f) -> p g f", f=fin),
                            in1=dcol[:].to_broadcast([P, Gg, fin]),
                            op=mybir.AluOpType.mult)
                        at = outp.tile([P, Gg * fout], f32, tag=f"A{tag}",
                                       name=f"A{tag}_{gi}")
                        nc.scalar.activation(
                            out=at[:], in_=S[:],
                            func=mybir.ActivationFunctionType.Relu)
                    # store: rows (t0+g)*P + p
                    nc.sync.dma_start(
                        out=out_dram[t0 * P:(t0 + Gg) * P, :].rearrange(
                            "(g p) f -> p g f", p=P),
                        in_=at[:])
                    t0 += Gg
                    col += ncols

            # =========== owner-side fetch gather ===========
            def fetch_phase(idx_d, table_ap, out_dram, f, tag):
                Ctot = idx_d.shape[1]
                GS = meta["GS"]
                col = 0
                while col < Ctot:
                    g = min(GS, Ctot - col)
                    idxt = idxp.tile([P, g], i32, tag=f"fidx{tag}",
                                     name=f"fidx{tag}_{col}")
                    nc.sync.dma_start(out=idxt[:], in_=idx_d[:, col:col + g])
                    gt = gathp.tile([P, g * f], f32, tag=f"fg{tag}",
                                    name=f"fg{tag}_{col}")
                    for j in range(g):
                        nc.gpsimd.indirect_dma_start(
                            out=gt[:, j * f:(j + 1) * f],
                            out_offset=None,
                            in_=table_ap,
                            in_offset=IndirectOffsetOnAxis(
                                ap=idxt[:, j:j + 1], axis=0),
                        )
                    # store rows col*P .. (col+g)*P  (flat j = col*128 + p)
                    nc.sync.dma_start(
                        out=out_dram[col * P:(col + g) * P, :].rearrange(
                            "(g p) f -> p g f", p=P),
                        in_=gt[:])
                    col += g

            # =========== build phase (x @ W * dis) ===========
            def build_phase(recv_ap, disr_d, out_dram, fin, fout, G, wblk,
                            nrows, tag):
                ntile = nrows // P
                t0 = 0
                while t0 < ntile:
                    Gg = min(G, ntile - t0)
                    xt = workp.tile([P, Gg * fin], f32, tag=f"x{tag}",
                                    name=f"x{tag}_{t0}")
                    nc.sync.dma_start(
                        out=xt[:],
                        in_=recv_ap[t0 * P:(t0 + Gg) * P, :].rearrange(
                            "(g p) f -> p g f", p=P))
                    pT = psumTp.tile([Gg * fin, P], f32, tag="pT",
                                     name=f"pTb{tag}_{t0}")
                    nc.tensor.transpose(out=pT[:], in_=xt[:], identity=ident[:])
                    ST = workp.tile([Gg * fin, P], f32, tag=f"STb{tag}",
                                    name=f"STb{tag}_{t0}")
                    nc.scalar.copy(out=ST[:], in_=pT[:])
                    pM = psumMp.tile([P, Gg * fout], f32, tag="pM",
                                     name=f"pMb{tag}_{t0}")
                    nc.tensor.matmul(out=pM[:], lhsT=ST[:],
                                     rhs=wblk[:Gg * fin, :Gg * fout],
                                     start=True, stop=True)
                    dcol = idxp.tile([P, Gg], f32, tag=f"db{tag}",
                                     name=f"db{tag}_{t0}")
                    nc.sync.dma_start(out=dcol[:], in_=disr_d[:, t0:t0 + Gg])
                    gt = outp.tile([P, Gg * fout], f32, tag=f"gb{tag}",
                                   name=f"gb{tag}_{t0}")
                    nc.vector.tensor_tensor(
                        out=gt[:].rearrange("p (g f) -> p g f", f=fout),
                        in0=pM[:].rearrange("p (g f) -> p g f", f=fout),
                        in1=dcol[:].to_broadcast([P, Gg, fout]),
                        op=mybir.AluOpType.mult)
                    nc.sync.dma_start(
                        out=out_dram[t0 * P:(t0 + Gg) * P, :].rearrange(
                            "(g p) f -> p g f", p=P),
                        in_=gt[:])
                    t0 += Gg

            # ================= the program =================
            def dump(src_ap, nrows, f):
                # DRAM->DRAM copy via SBUF bounce, 128-row tiles
                for t0 in range(0, nrows // P, 8):
                    g = min(8, nrows // P - t0)
                    bt = outp.tile([P, g * f], f32, tag="dump", name=f"dump_{t0}")
                    nc.sync.dma_start(out=bt[:], in_=src_ap[t0 * P:(t0 + g) * P, :]
                                      .rearrange("(g p) f -> p g f", p=P))
                    nc.sync.dma_start(out=dbg[t0 * P:(t0 + g) * P, :]
                                      .rearrange("(g p) f -> p g f", p=P), in_=bt[:])

            _stopped = False
            if stop_after == "X0":
                _stopped = True
                dump(x0p[:, :], P, 4)
            # L1 agg -> A1sh
            if not _stopped:
                agg_phase(L1, idx1, wg1, dis1d, x0p[:, :], A1sh[:], w1b, "1")
            if not _stopped and stop_after == "L1":
                _stopped = True
                dump(A1sh[:], L1["shardP"], 32)
            # C1: AllGather A1
            if not _stopped:
                nc.gpsimd.collective_compute(
                    "AllGather", mybir.AluOpType.bypass, replica_groups=rg,
                    ins=[A1sh[:]], outs=[A1f[:]])
            if not _stopped and stop_after == "C1":
                _stopped = True
                dump(A1f[:], NC * L1["shardP"], 32)
            # L2 agg -> A2sh
            if not _stopped:
                agg_phase(L2, idx2, wg2, dis2d, A1f[:], A2sh[:], w2b, "2")
            if not _stopped and stop_after == "L2":
                _stopped = True
                dump(A2sh[:], L2["shardP"], 64)
            # C2: owner gather + A2A
            if not _stopped:
                fetch_phase(sidx3, A2sh[:], send3[:], 64, "3")
            if not _stopped and stop_after == "S3":
                _stopped = True
                dump(send3[:], NC * meta["B3"], 64)
            if not _stopped:
                nc.gpsimd.collective_compute(
                    "AllToAll", mybir.AluOpType.bypass, replica_groups=rg,
                    ins=[send3[:]], outs=[recv3[:]])
            if not _stopped and stop_after == "C2":
                _stopped = True
                dump(recv3[:], NC * meta["B3"], 64)
            _stopped2 = False
            if not _stopped:
                # g3 build (+ zero tile)
                build_phase(recv3[:], dis3r, g3sh[:], 64, 32,
                            meta["b3"]["G"], w3b, NC * meta["B3"], "3")
                zt3 = outp.tile([P, 32], f32, tag="zt", name="zt3")
                nc.vector.memset(zt3[:], 0.0)
                nc.sync.dma_start(out=g3sh[NC * meta["B3"]:NC * meta["B3"] + P, :]
                                  .rearrange("(a p) f -> p a f", p=P),
                                  in_=zt3[:])
                if stop_after == "B3":
                    _stopped2 = True
                    dump(g3sh[:], meta["S3"], 32)
                # C3: AllGather g3
                nc.gpsimd.collective_compute(
                    "AllGather", mybir.AluOpType.bypass, replica_groups=rg,
                    ins=[g3sh[:]], outs=[g3f[:]])
                if stop_after == "C3":
                    _stopped2 = True
                    dump(g3f[:], NC * meta["S3"], 32)
                # L3 agg -> A3sh
                agg_phase(L3, idx3, None, dis3d, g3f[:], A3sh[:], None, "L3")
                if stop_after == "L3":
                    _stopped2 = True
                    dump(A3sh[:], L3["shardP"], 32)
                # C4: owner gather + A2A
                fetch_phase(sidx4, A3sh[:], send4[:], 32, "4")
                if stop_after == "S4":
                    _stopped2 = True
                    dump(send4[:], NC * meta["B4"], 32)
                nc.gpsimd.collective_compute(
                    "AllToAll", mybir.AluOpType.bypass, replica_groups=rg,
                    ins=[send4[:]], outs=[recv4[:]])
                if stop_after == "C4":
                    _stopped2 = True
                    dump(recv4[:], NC * meta["B4"], 32)
                # g4 build (+ zero tile)
                build_phase(recv4[:], dis4r, g4sh[:], 32, 4,
                            meta["b4"]["G"], w4b, NC * meta["B4"], "4")
                zt4 = outp.tile([P, 4], f32, tag="zt4", name="zt4")
                nc.vector.memset(zt4[:], 0.0)
                nc.sync.dma_start(out=g4sh[NC * meta["B4"]:NC * meta["B4"] + P, :]
                                  .rearrange("(a p) f -> p a f", p=P),
                                  in_=zt4[:])
                if stop_after == "B4":
                    _stopped2 = True
                    dump(g4sh[:], meta["S4"], 4)
                # C5: AllGather g4
                nc.gpsimd.collective_compute(
                    "AllGather", mybir.AluOpType.bypass, replica_groups=rg,
                    ins=[g4sh[:]], outs=[g4f[:]])
                if stop_after == "C5":
                    _stopped2 = True
                    dump(g4f[:], NC * meta["S4"], 4)
                # L4 agg -> A4sh
                agg_phase(L4, idx4, None, dis4d, g4f[:], A4sh[:], None, "L4")
                if stop_after == "L4":
                    _stopped2 = True
                    dump(A4sh[:], L4["shardP"], 4)
                # Final: owner gather by unpool4 into output
                fetch_phase(fidx, A4sh[:], outbuf[:, :], 4, "F")

    nc.finalize()
    return nc


# ----------------------------------------------------------------------------
# PJRT runner (persistent compiled callable, device-resident inputs)
# ----------------------------------------------------------------------------
import numpy as np, time
import jax
import jax.numpy as jnp
from jax.sharding import Mesh, PartitionSpec, NamedSharding
from jax.experimental.shard_map import shard_map
from concourse import mybir
from concourse.bass2jax import _bass_exec_p, partition_id_tensor, install_neuronx_cc_hook


def make_runner(nc, n_cores=8):
    install_neuronx_cc_hook()
    partition_name = nc.partition_id_tensor.name if nc.partition_id_tensor else None
    in_names, out_names, out_avals = [], [], []
    for alloc in nc.m.functions[0].allocations:
        if not isinstance(alloc, mybir.MemoryLocationSet):
            continue
        name = alloc.memorylocations[0].name
        if alloc.kind == "ExternalInput":
            if name != partition_name:
                in_names.append(name)
        elif alloc.kind == "ExternalOutput":
            out_names.append(name)
            out_avals.append(jax.core.ShapedArray(
                tuple(alloc.tensor_shape), mybir.dt.np(alloc.dtype)))
    n_params = len(in_names)
    all_in_names = list(in_names) + list(out_names)
    if partition_name is not None:
        all_in_names.append(partition_name)

    def _body(*args):
        operands = list(args)
        if partition_name is not None:
            operands.append(partition_id_tensor())
        outs = _bass_exec_p.bind(
            *operands,
            out_avals=tuple(out_avals), in_names=tuple(all_in_names),
            out_names=tuple(out_names), lowering_input_output_aliases=(),
            sim_require_finite=False, sim_require_nnan=False, nc=nc)
        return tuple(outs)

    devices = jax.devices()[:n_cores]
    mesh = Mesh(np.asarray(devices), ("core",))
    n_outs = len(out_avals)
    in_specs = (PartitionSpec("core"),) * (n_params + n_outs)
    out_specs = (PartitionSpec("core"),) * len(out_names)
    sharded = jax.jit(shard_map(_body, mesh=mesh, in_specs=in_specs,
                                out_specs=out_specs, check_rep=False),
                      keep_unused=True)
    sharding = NamedSharding(mesh, PartitionSpec("core"))

    state = {}

    def prepare(in_maps):
        per_core = [[np.asarray(m[name]) for name in in_names] for m in in_maps]
        concat_in = [np.concatenate([per_core[c][i] for c in range(n_cores)], axis=0)
                     for i in range(n_params)]
        zeros = [np.zeros((n_cores * av.shape[0], *av.shape[1:]), av.dtype)
                 for av in out_avals]
        state["dev_in"] = [jax.device_put(a, sharding) for a in concat_in + zeros]
        jax.block_until_ready(state["dev_in"])

    def run():
        out = jax.block_until_ready(sharded(*state["dev_in"]))
        return out

    def fetch(out_arrs):
        return [
            {name: np.asarray(out_arrs[i]).reshape(n_cores, *out_avals[i].shape)[c]
             for i, name in enumerate(out_names)}
            for c in range(n_cores)
        ]

    return prepare, run, fetch


# ----------------------------------------------------------------------------
# public entry point
# ----------------------------------------------------------------------------
_CACHE = {}


def kernel(**inputs):
    import numpy as np
    for b in ("b1", "b2", "b3", "b4"):
        if b in inputs:
            assert not np.asarray(inputs[b]).any(), (
                "this kernel build assumes zero biases (per problem spec)")
    meta, in_maps, reassemble = prep(inputs)
    key = "k"
    if key not in _CACHE:
        nc = build_kernel(meta)
        _CACHE[key] = make_runner(nc)
    prepare, run, fetch = _CACHE[key]
    prepare(in_maps)
    outs = fetch(run())
    return reassemble(outs).astype(np.float32)



# revision 3
# speedup vs baseline: 1.3960x; 1.3960x over previous
"""Trainium2 Bass kernel v2 for the 4-layer GCN + mesh-unpool network,
8 NeuronCores, dst-sharded graph parallelism.

vs v1 (baseline): per-column indirect DMA gathers (994ns Q7 emission per
128 rows -> ~14ms) are replaced by bulk InstDMAGatherAnt gathers
(~8k rows per instruction, round-robin over 4 SWDGE queues), with bf16
tables packed K nodes per 256B+ row (pure reinterpretation of the
row-major [V, F] table).  W3/W4 are pushed through the aggregation
(linearity), eliminating both AllToAll exchanges and the fetch/build
phases; unpools become host-side index composition.

Per layer l (dst shard on each core): padded-CSR slot grids over
in-degree-sorted nodes; slots gathered from the bf16 table (row idx//K,
sub-node idx%K selected by a DVE mask), weighted by dis_l[src] (and
killed for pad slots), segment-reduced, scaled by dis_l[dst], matmul'd
(W1/W2; identity collapse for L3/L4 which gather pre-multiplied
tables), relu'd, stored bf16, AllGathered into the next table.
"""
import sys
sys.path.insert(0, "/opt/trn_rl_repo")

import numpy as np
import ml_dtypes

NC = 8
P = 128
NQ = 4            # SWDGE queues
MAXIDX = 8192     # idx per dma_gather instruction (HW-validated)


def pad_to(x, m):
    return (x + m - 1) // m * m


# ----------------------------------------------------------------------------
# host-side planning
# ----------------------------------------------------------------------------

def make_dis(edge_index, n):
    deg = np.bincount(edge_index[1], minlength=n).astype(np.float64) + 1.0
    return (1.0 / np.sqrt(deg)).astype(np.float32)


def plan_agg(dst_old, slot_row, n, self_row, slot_weight, self_weight, G,
             col_budget=64):
    """Padded-CSR plan (degree-sorted, program-unified across cores).

    Returns per-core idx [P, Ctot] int32 (table row ids), w [P, Ctot]
    fp32, groups [(Gg, kt)], sigma (old id -> global new row), dis-dst
    columns come separately.
    """
    shard = n // NC
    shardP = pad_to(shard, P)
    ntiles = shardP // P
    c_of = dst_old // shard

    percore = []
    for c in range(NC):
        m = c_of == c
        dl = dst_old[m] - c * shard
        deg = np.bincount(dl, minlength=shard)
        perm = np.argsort(-deg, kind="stable")
        inv = np.empty_like(perm); inv[perm] = np.arange(shard)
        percore.append((m, dl, deg, perm, inv))

    def span_kt(lo_t, Gg):
        lo, hi = lo_t * P, min((lo_t + Gg) * P, shard)
        kt = 0
        for c in range(NC):
            deg_new = percore[c][2][percore[c][3]]
            if hi > lo:
                kt = max(kt, int(deg_new[lo:hi].max()))
        return kt + 1

    groups = []
    t = 0
    while t < ntiles:
        Gg = min(G, ntiles - t)
        kt = span_kt(t, Gg)
        while Gg > 1 and Gg * kt > col_budget:
            Gg = max(1, min(Gg - 1, col_budget // kt))
            kt = span_kt(t, Gg)
        groups.append((Gg, kt))
        t += Gg

    colbase = np.zeros(ntiles + 1, np.int64)
    kts_tile = []
    for (Gg, kt) in groups:
        kts_tile += [kt] * Gg
    for t in range(ntiles):
        colbase[t + 1] = colbase[t] + kts_tile[t]
    Ctot = int(colbase[-1])

    out = {"groups": groups, "idx": [], "w": [], "perm": [],
           "shardP": shardP, "Ctot": Ctot}
    sigma = np.empty(n, np.int64)
    for c in range(NC):
        m, dl, deg, perm, inv = percore[c]
        sigma[c * shard:(c + 1) * shard] = c * shardP + inv
        sr = slot_row[m]
        sw = slot_weight[m]
        nd = inv[dl]
        order = np.argsort(nd, kind="stable")
        nd_s = nd[order]; sr_s = sr[order]
        deg_new = deg[perm]
        starts = np.zeros(shard + 1, np.int64)
        np.cumsum(deg_new, out=starts[1:])
        idx = np.zeros((P, Ctot), np.int64)
        w = np.zeros((P, Ctot), np.float32)
        r = np.arange(len(nd_s)) - starts[nd_s]
        pp_ = nd_s % P
        tt_ = nd_s // P
        cols = colbase[tt_] + r
        idx[pp_, cols] = sr_s
        w[pp_, cols] = sw[order]
        v = np.arange(shard)
        scols = colbase[v // P] + deg_new[v]
        idx[v % P, scols] = self_row[perm + c * shard]
        w[v % P, scols] = self_weight[perm + c * shard]
        out["idx"].append(idx)
        out["w"].append(w)
        out["perm"].append(perm)
    out["sigma"] = sigma
    return out


def tile_cols(vec_percore, shardP):
    outs = []
    for v in vec_percore:
        a = np.zeros(shardP, np.float32)
        a[:len(v)] = v
        outs.append(a.reshape(shardP // P, P).T.copy())
    return outs


def wrap16(idx_cols):
    """[P, C] column-major slot grid -> dma_gather idx layout.

    Slot i (= c*128 + p) must sit at (partition i%16, col i//16),
    replicated across the 8 16-partition groups. Returns [P, C*8] int16.
    """
    Pp, C = idx_cols.shape
    flat = idx_cols.T.reshape(-1)              # slot i order
    n = flat.shape[0]
    w = flat.reshape(n // 16, 16).T            # [16, n/16]
    return np.tile(w, (8, 1)).astype(np.int16) # [128, n/16]


def split_kq(idx, K):
    return (idx // K).astype(np.int64), (idx % K).astype(np.float32)


def prep(inputs, G1=8, G2=2, G3=4, G4=8, GF=8):
    x = np.asarray(inputs["x"], np.float32)
    W = [np.asarray(inputs[f"W{i}"], np.float32) for i in (1, 2, 3, 4)]
    ei = [np.asarray(inputs[f"edge_index{i}"]).astype(np.int64) for i in range(4)]
    u = [np.asarray(inputs[f"unpool{i}"]).astype(np.int64) for i in (1, 2, 3, 4)]
    n = [x.shape[0], len(u[0]), len(u[1]), len(u[2])]
    nout = len(u[3])
    dis = [make_dis(ei[l], n[l]) for l in range(4)]

    # L1: table rows = original x ids (dis1 folded into table)
    L1 = plan_agg(ei[0][1], ei[0][0], n[0], self_row=np.arange(n[0]), G=G1,
                  slot_weight=np.ones(ei[0].shape[1], np.float32),
                  self_weight=np.ones(n[0], np.float32))
    s1 = L1["sigma"]
    # L2: rows in T2 (=A1, global new order), composed via u1
    L2 = plan_agg(ei[1][1], s1[u[0][ei[1][0]]], n[1], self_row=s1[u[0]], G=G2,
                  slot_weight=dis[1][ei[1][0]], self_weight=dis[1])
    s2 = L2["sigma"]
    # L3: rows in T3 (=B2 = A2@W3), composed via u2
    L3 = plan_agg(ei[2][1], s2[u[1][ei[2][0]]], n[2], self_row=s2[u[1]], G=G3,
                  slot_weight=dis[2][ei[2][0]], self_weight=dis[2])
    s3 = L3["sigma"]
    # L4: rows in T4 (=B3 = A3@W4), composed via u3
    L4 = plan_agg(ei[3][1], s3[u[2][ei[3][0]]], n[3], self_row=s3[u[2]], G=G4,
                  slot_weight=dis[3][ei[3][0]], self_weight=dis[3])
    s4 = L4["sigma"]

    # dis-dst columns (new-local order per core)
    dis_dst = []
    for l, L in enumerate((L1, L2, L3, L4)):
        sh = n[l] // NC
        dis_dst.append(tile_cols(
            [dis[l][c * sh + L["perm"][c]] for c in range(NC)], L["shardP"]))

    # per-layer packing K (nodes per 256B+ table row, bf16)
    Ks = {"1": 32, "2": 4, "3": 8, "4": 32, "F": 32}
    Fs = {"1": 4, "2": 32, "3": 32, "4": 4, "F": 4}

    # final gather: per-core slice of u4 -> rows s4[u4]
    shf = nout // NC
    shfP = pad_to(shf, P)
    CF = shfP // P
    fidx, fq, fw = [], [], []
    for c in range(NC):
        rows = s4[u[3][c * shf:(c + 1) * shf]]
        a = np.zeros(shfP, np.int64)
        a[:shf] = rows
        grid = a.reshape(CF, P).T  # [P, CF] column-major slots
        fidx.append(grid)

    meta = dict(
        n=n, nout=nout, shf=shf, shfP=shfP, CF=CF, Ks=Ks, Fs=Fs,
        L1=dict(groups=L1["groups"], shardP=L1["shardP"], C=L1["Ctot"],
                fout=32, wmat="w1", relu=True),
        L2=dict(groups=L2["groups"], shardP=L2["shardP"], C=L2["Ctot"],
                fout=64, wmat="w2", relu=True),
        L3=dict(groups=L3["groups"], shardP=L3["shardP"], C=L3["Ctot"],
                fout=32, wmat=None, relu=True),
        L4=dict(groups=L4["groups"], shardP=L4["shardP"], C=L4["Ctot"],
                fout=4, wmat=None, relu=True),
        T1_rows=pad_to(n[0], Ks["1"]) // Ks["1"],
        T2_rows=NC * L1["shardP"] // Ks["2"],
        T3_rows=NC * L2["shardP"] // Ks["3"],
        T4_rows=NC * L3["shardP"] // Ks["4"],
        T5_rows=NC * L4["shardP"] // Ks["F"],
    )

    # ---- per-core inputs ----
    # T1: x * dis1, padded to 4 feats, bf16, row-major [V,4] (viewed packed)
    V1 = pad_to(n[0], Ks["1"])
    T1 = np.zeros((V1, 4), np.float32)
    T1[:n[0], :3] = x * dis[0][:, None]
    T1 = T1.astype(ml_dtypes.bfloat16)

    W1p = np.zeros((4, 32), np.float32); W1p[:3] = W[0]
    W4p = np.zeros((32, 4), np.float32); W4p[:, :3] = W[3]

    def blkdiag(Wm, G):
        fi, fo = Wm.shape
        B = np.zeros((G * fi, G * fo), np.float32)
        for g in range(G):
            B[g * fi:(g + 1) * fi, g * fo:(g + 1) * fo] = Wm
        return B

    ident = np.eye(P, dtype=np.float32)
    qio = np.tile(np.arange(32, dtype=np.float32)[None, :], (128, 1)).astype(ml_dtypes.bfloat16)

    in_maps = []
    for c in range(NC):
        i1, q1 = split_kq(L1["idx"][c], Ks["1"])
        i2, q2 = split_kq(L2["idx"][c], Ks["2"])
        i3, q3 = split_kq(L3["idx"][c], Ks["3"])
        i4, q4 = split_kq(L4["idx"][c], Ks["4"])
        iF, qF = split_kq(fidx[c], Ks["F"])
        m = {
            "T1": T1, "ident": ident, "qio": qio,
            "w1blk": blkdiag(W1p, G1), "w2blk": blkdiag(W[1], G2),
            "w3blk": blkdiag(W[2], G2),
            "idx1": wrap16(i1), "q1": q1.astype(ml_dtypes.bfloat16),
            "wg1": L1["w"][c].astype(ml_dtypes.bfloat16), "dis1d": dis_dst[0][c],
            "idx2": wrap16(i2), "q2": q2.astype(ml_dtypes.bfloat16),
            "wg2": L2["w"][c].astype(ml_dtypes.bfloat16), "dis2d": dis_dst[1][c],
            "idx3": wrap16(i3), "q3": q3.astype(ml_dtypes.bfloat16),
            "wg3": L3["w"][c].astype(ml_dtypes.bfloat16), "dis3d": dis_dst[2][c],
            "idx4": wrap16(i4), "q4": q4.astype(ml_dtypes.bfloat16),
            "wg4": L4["w"][c].astype(ml_dtypes.bfloat16), "dis4d": dis_dst[3][c],
            "w4blk": blkdiag(W4p, G3),
            "idxF": wrap16(iF), "qF": qF.astype(ml_dtypes.bfloat16),
        }
        in_maps.append(m)

    def reassemble(outs):
        res = np.concatenate([outs[c]["outbuf"][:shf, :3] for c in range(NC)],
                             axis=0)
        return np.ascontiguousarray(res.astype(np.float32))

    return meta, in_maps, reassemble


# ----------------------------------------------------------------------------
# device kernel
# ----------------------------------------------------------------------------

def build_kernel(meta):
    import concourse.bass as bass
    import concourse.mybir as mybir
    from concourse.bacc import Bacc
    from concourse.tile import TileContext
    from concourse import library_config

    f32 = mybir.dt.float32
    bf16 = mybir.dt.bfloat16
    i16 = mybir.dt.int16
    n = meta["n"]
    Ks, Fs = meta["Ks"], meta["Fs"]

    nc = Bacc("TRN2", target_bir_lowering=False, debug=False, num_devices=NC,
              num_swdge_queues=NQ)

    T1_d = nc.dram_tensor("T1", [meta["T1_rows"] * Ks["1"], 4], bf16,
                          kind="ExternalInput")
    ident_d = nc.dram_tensor("ident", [P, P], f32, kind="ExternalInput")
    qio_d = nc.dram_tensor("qio", [P, 32], bf16, kind="ExternalInput")
    w1blk_d = nc.dram_tensor("w1blk", [8 * 4, 8 * 32], f32, kind="ExternalInput")
    w2blk_d = nc.dram_tensor("w2blk", [2 * 32, 2 * 64], f32, kind="ExternalInput")
    w3blk_d = nc.dram_tensor("w3blk", [2 * 64, 2 * 32], f32, kind="ExternalInput")
    w4blk_d = nc.dram_tensor("w4blk", [4 * 32, 4 * 4], f32, kind="ExternalInput")

    L1m, L2m, L3m, L4m = meta["L1"], meta["L2"], meta["L3"], meta["L4"]

    def grid_in(name, C, dt, scale=8):
        # idx grids are [P, C*128/16] int16; q/w grids [P, C]
        return nc.dram_tensor(name, [P, C * scale], dt, kind="ExternalInput")

    idx1 = grid_in("idx1", L1m["C"], i16); q1 = grid_in("q1", L1m["C"], bf16, 1)
    wg1 = grid_in("wg1", L1m["C"], bf16, 1)
    dis1d = grid_in("dis1d", L1m["shardP"] // P, f32, 1)
    idx2 = grid_in("idx2", L2m["C"], i16); q2 = grid_in("q2", L2m["C"], bf16, 1)
    wg2 = grid_in("wg2", L2m["C"], bf16, 1)
    dis2d = grid_in("dis2d", L2m["shardP"] // P, f32, 1)
    idx3 = grid_in("idx3", L3m["C"], i16); q3 = grid_in("q3", L3m["C"], bf16, 1)
    wg3 = grid_in("wg3", L3m["C"], bf16, 1)
    dis3d = grid_in("dis3d", L3m["shardP"] // P, f32, 1)
    idx4 = grid_in("idx4", L4m["C"], i16); q4 = grid_in("q4", L4m["C"], bf16, 1)
    wg4 = grid_in("wg4", L4m["C"], bf16, 1)
    dis4d = grid_in("dis4d", L4m["shardP"] // P, f32, 1)
    idxF = grid_in("idxF", meta["CF"], i16); qF = grid_in("qF", meta["CF"], bf16, 1)

    outbuf = nc.dram_tensor("outbuf", [meta["shfP"], 4], f32,
                            kind="ExternalOutput")
    rg = [list(range(NC))]

    qctr = [0]

    def next_q():
        q = qctr[0] % NQ
        qctr[0] += 1
        return q

    with TileContext(nc) as tc:
        nc.gpsimd.load_library(library_config.mlp)
        with (
            tc.tile_pool(name="dramp", bufs=1, space="DRAM") as dramp,
            tc.tile_pool(name="consts", bufs=1) as constp,
            tc.tile_pool(name="idxp", bufs=6) as idxp,
            tc.tile_pool(name="gath", bufs=4) as gathp,
            tc.tile_pool(name="work", bufs=4) as workp,
            tc.tile_pool(name="outp", bufs=3) as outp,
            tc.tile_pool(name="psumT", bufs=2, space="PSUM") as psumTp,
            tc.tile_pool(name="psumM", bufs=2, space="PSUM") as psumMp,
        ):
            # persistent DRAM intermediates (bf16 tables)
            A1sh = dramp.tile([L1m["shardP"], 32], bf16)
            T2f = dramp.tile([NC * L1m["shardP"], 32], bf16, addr_space="Shared")
            B2sh = dramp.tile([L2m["shardP"], 32], bf16)
            T3f = dramp.tile([NC * L2m["shardP"], 32], bf16, addr_space="Shared")
            B3sh = dramp.tile([L3m["shardP"], 4], bf16)
            T4f = dramp.tile([NC * L3m["shardP"], 4], bf16, addr_space="Shared")
            A4sh = dramp.tile([L4m["shardP"], 4], bf16)
            T5f = dramp.tile([NC * L4m["shardP"], 4], bf16, addr_space="Shared")

            # constants
            ident = constp.tile([P, P], f32)
            nc.sync.dma_start(out=ident[:], in_=ident_d[:, :])
            qio16 = constp.tile([P, 32], bf16)
            nc.sync.dma_start(out=qio16[:], in_=qio_d[:, :])
            w1b = constp.tile([32, 8 * 32], f32)
            nc.sync.dma_start(out=w1b[:], in_=w1blk_d[:, :])
            w2b = constp.tile([64, 2 * 64], f32)
            nc.sync.dma_start(out=w2b[:], in_=w2blk_d[:, :])
            w3b = constp.tile([2 * 64, 2 * 32], f32)
            nc.sync.dma_start(out=w3b[:], in_=w3blk_d[:, :])
            w4b = constp.tile([4 * 32, 4 * 4], f32)
            nc.sync.dma_start(out=w4b[:], in_=w4blk_d[:, :])

            def agg_phase(lm, K, F, idx_d, q_d, w_d, disd_d, table_view,
                          wblk, out_dram, out_f, second=None, tag=""):
                """One GCN layer aggregation over the padded-CSR grid.

                table_view: DRAM AP [rows, K*F] bf16.
                wblk: None -> identity collapse (gathered feats are final);
                      else (tile, Gfi, Gfo) block-diag matmul after reduce.
                second: optional (w3b-style tile, fi, fo) fused second
                      matmul producing out rows (for B2).
                """
                t0 = 0
                col = 0
                for gi, (Gg, kt) in enumerate(lm["groups"]):
                    ncols = Gg * kt
                    nslots = ncols * P
                    # gathered tile [P, ncols, K*F] bf16 (slot i -> i%128,i//128)
                    gt = gathp.tile([P, ncols * K * F], bf16, tag="g",
                                    name=f"g{tag}_{gi}")
                    # gather in chunks of MAXIDX slots (=MAXIDX/128 cols)
                    ccols = MAXIDX // P
                    for c0 in range(0, ncols, ccols):
                        cw = min(ccols, ncols - c0)
                        idxt = idxp.tile([P, cw * P // 16], i16, tag="i",
                                         name=f"i{tag}_{gi}_{c0}")
                        nc.sync.dma_start(
                            out=idxt[:],
                            in_=idx_d[:, (col + c0) * 8:(col + c0 + cw) * 8])
                        nc.gpsimd.dma_gather(
                            out_ap=gt[:, c0 * K * F:(c0 + cw) * K * F]
                                .rearrange("p (m e) -> p m e", e=K * F),
                            in_ap=table_view,
                            idxs_ap=idxt[:],
                            num_idxs=cw * P, num_idxs_reg=cw * P,
                            elem_size=K * F,
                            single_packet=False, queue_num=next_q())
                    # subpos mask: ind[p, c, K] = (q[p,c] == qio[K])
                    qt = idxp.tile([P, ncols], bf16, tag="q",
                                   name=f"q{tag}_{gi}")
                    nc.sync.dma_start(out=qt[:], in_=q_d[:, col:col + ncols])
                    wt = idxp.tile([P, ncols], bf16, tag="w",
                                   name=f"w{tag}_{gi}")
                    nc.sync.dma_start(out=wt[:], in_=w_d[:, col:col + ncols])
                    ind = workp.tile([P, ncols * K], bf16, tag="n",
                                     name=f"n{tag}_{gi}")
                    nc.vector.tensor_tensor(
                        out=ind[:].rearrange("p (c k) -> p c k", k=K),
                        in0=qt[:].to_broadcast([P, ncols, K]),
                        in1=qio16[:, :K].rearrange("p (o k) -> p o k", o=1)
                            .to_broadcast([P, ncols, K]),
                        op=mybir.AluOpType.is_equal)
                    # fold w: mw = ind * w
                    nc.vector.tensor_tensor(
                        out=ind[:].rearrange("p (c k) -> p c k", k=K),
                        in0=ind[:].rearrange("p (c k) -> p c k", k=K),
                        in1=wt[:].to_broadcast([P, ncols, K]),
                        op=mybir.AluOpType.mult)
                    # apply to gathered rows
                    nc.vector.tensor_tensor(
                        out=gt[:].rearrange("p (c k f) -> p c k f", k=K, f=F),
                        in0=gt[:].rearrange("p (c k f) -> p c k f", k=K, f=F),
                        in1=ind[:].rearrange("p (c k) -> p c k", k=K)
                            .to_broadcast([P, ncols, K, F]),
                        op=mybir.AluOpType.mult)
                    # segment reduce over (kt*K) keeping F
                    S = workp.tile([P, Gg * F], f32, tag="S",
                                   name=f"S{tag}_{gi}")
                    nc.vector.tensor_reduce(
                        out=S[:].rearrange("p (g f) -> p g f", f=F),
                        in_=gt[:].rearrange("p (g x f) -> p g f x",
                                            g=Gg, x=kt * K),
                        axis=mybir.AxisListType.X, op=mybir.AluOpType.add)
                    # dis_dst scale
                    dcol = idxp.tile([P, Gg], f32, tag="d",
                                     name=f"d{tag}_{gi}")
                    nc.sync.dma_start(out=dcol[:], in_=disd_d[:, t0:t0 + Gg])
                    nc.vector.tensor_tensor(
                        out=S[:].rearrange("p (g f) -> p g f", f=F),
                        in0=S[:].rearrange("p (g f) -> p g f", f=F),
                        in1=dcol[:].to_broadcast([P, Gg, F]),
                        op=mybir.AluOpType.mult)
                    if wblk is not None:
                        wtile, gfi, gfo = wblk
                        pT = psumTp.tile([Gg * gfi, P], f32, tag="pT",
                                         name=f"pT{tag}_{gi}")
                        nc.tensor.transpose(out=pT[:], in_=S[:],
                                            identity=ident[:])
                        ST = workp.tile([Gg * gfi, P], f32, tag="ST",
                                        name=f"ST{tag}_{gi}")
                        nc.scalar.copy(out=ST[:], in_=pT[:])
                        pM = psumMp.tile([P, Gg * gfo], f32, tag="pM",
                                         name=f"pM{tag}_{gi}")
                        nc.tensor.matmul(out=pM[:], lhsT=ST[:],
                                         rhs=wtile[:Gg * gfi, :Gg * gfo],
                                         start=True, stop=True)
                        src_ap = pM
                        fo = gfo
                    else:
                        src_ap = S
                        fo = F
                    at = outp.tile([P, Gg * fo], f32, tag="A",
                                   name=f"A{tag}_{gi}")
                    nc.scalar.activation(
                        out=at[:], in_=src_ap[:],
                        func=mybir.ActivationFunctionType.Relu)
                    if second is None:
                        ab = outp.tile([P, Gg * fo], bf16, tag="Ab",
                                       name=f"Ab{tag}_{gi}")
                        nc.vector.tensor_copy(out=ab[:], in_=at[:])
                        nc.sync.dma_start(
                            out=out_dram[t0 * P:(t0 + Gg) * P, :].rearrange(
                                "(g p) f -> p g f", p=P),
                            in_=ab[:])
                    else:
                        stile, sfi, sfo = second
                        pT2 = psumTp.tile([Gg * sfi, P], f32, tag="pT",
                                          name=f"pT2{tag}_{gi}")
                        nc.tensor.transpose(out=pT2[:], in_=at[:],
                                            identity=ident[:])
                        ST2 = workp.tile([Gg * sfi, P], f32, tag="ST2",
                                         name=f"ST2{tag}_{gi}")
                        nc.scalar.copy(out=ST2[:], in_=pT2[:])
                        pM2 = psumMp.tile([P, Gg * sfo], f32, tag="pM",
                                          name=f"pM2{tag}_{gi}")
                        nc.tensor.matmul(out=pM2[:], lhsT=ST2[:],
                                         rhs=stile[:Gg * sfi, :Gg * sfo],
                                         start=True, stop=True)
                        ab = outp.tile([P, Gg * sfo], bf16, tag="Ab",
                                       name=f"Ab{tag}_{gi}")
                        nc.vector.tensor_copy(out=ab[:], in_=pM2[:])
                        nc.sync.dma_start(
                            out=out_dram[t0 * P:(t0 + Gg) * P, :].rearrange(
                                "(g p) f -> p g f", p=P),
                            in_=ab[:])
                    t0 += Gg
                    col += ncols

            # ---------------- L1 ----------------
            agg_phase(L1m, Ks["1"], Fs["1"], idx1, q1, wg1, dis1d,
                      T1_d[:, :].rearrange("(r k) f -> r (k f)", k=Ks["1"]),
                      (w1b, 4, 32), A1sh[:], 32, tag="1")
            nc.gpsimd.collective_compute(
                "AllGather", mybir.AluOpType.bypass, replica_groups=rg,
                ins=[A1sh[:]], outs=[T2f[:]])
            # ---------------- L2 (fused B2 = relu(...)@W3) ----------------
            agg_phase(L2m, Ks["2"], Fs["2"], idx2, q2, wg2, dis2d,
                      T2f[:].rearrange("(r k) f -> r (k f)", k=Ks["2"]),
                      (w2b, 32, 64), B2sh[:], 32, second=(w3b, 64, 32),
                      tag="2")
            nc.gpsimd.collective_compute(
                "AllGather", mybir.AluOpType.bypass, replica_groups=rg,
                ins=[B2sh[:]], outs=[T3f[:]])
            # ---------------- L3 (identity collapse; B3 = relu@W4) --------
            agg_phase(L3m, Ks["3"], Fs["3"], idx3, q3, wg3, dis3d,
                      T3f[:].rearrange("(r k) f -> r (k f)", k=Ks["3"]),
                      None, B3sh[:], 4, second=(w4b, 32, 4), tag="3")
            nc.gpsimd.collective_compute(
                "AllGather", mybir.AluOpType.bypass, replica_groups=rg,
                ins=[B3sh[:]], outs=[T4f[:]])
            # ---------------- L4 ----------------
            agg_phase(L4m, Ks["4"], Fs["4"], idx4, q4, wg4, dis4d,
                      T4f[:].rearrange("(r k) f -> r (k f)", k=Ks["4"]),
                      None, A4sh[:], 4, tag="4")
            nc.gpsimd.collective_compute(
                "AllGather", mybir.AluOpType.bypass, replica_groups=rg,
                ins=[A4sh[:]], outs=[T5f[:]])
            # ---------------- final unpool gather ----------------
            KF, FF = Ks["F"], Fs["F"]
            CF = meta["CF"]
            ccols = MAXIDX // P
            for c0 in range(0, CF, ccols):
                cw = min(ccols, CF - c0)
                idxt = idxp.tile([P, cw * 8], i16, tag="i", name=f"iF_{c0}")
                nc.sync.dma_start(out=idxt[:], in_=idxF[:, c0 * 8:(c0 + cw) * 8])
                gt = gathp.tile([P, cw * KF * FF], bf16, tag="g",
                                name=f"gF_{c0}")
                nc.gpsimd.dma_gather(
                    out_ap=gt[:].rearrange("p (m e) -> p m e", e=KF * FF),
                    in_ap=T5f[:].rearrange("(r k) f -> r (k f)", k=KF),
                    idxs_ap=idxt[:],
                    num_idxs=cw * P, num_idxs_reg=cw * P, elem_size=KF * FF,
                    single_packet=False, queue_num=next_q())
                qt = idxp.tile([P, cw], bf16, tag="q", name=f"qF_{c0}")
                nc.sync.dma_start(out=qt[:], in_=qF[:, c0:c0 + cw])
                ind = workp.tile([P, cw * KF], bf16, tag="n", name=f"nF_{c0}")
                nc.vector.tensor_tensor(
                    out=ind[:].rearrange("p (c k) -> p c k", k=KF),
                    in0=qt[:].to_broadcast([P, cw, KF]),
                    in1=qio16[:, :KF].rearrange("p (o k) -> p o k", o=1)
                        .to_broadcast([P, cw, KF]),
                    op=mybir.AluOpType.is_equal)
                nc.vector.tensor_tensor(
                    out=gt[:].rearrange("p (c k f) -> p c k f", k=KF, f=FF),
                    in0=gt[:].rearrange("p (c k f) -> p c k f", k=KF, f=FF),
                    in1=ind[:].rearrange("p (c k) -> p c k", k=KF)
                        .to_broadcast([P, cw, KF, FF]),
                    op=mybir.AluOpType.mult)
                S = outp.tile([P, cw * FF], f32, tag="A", name=f"SF_{c0}")
                nc.vector.tensor_reduce(
                    out=S[:].rearrange("p (c f) -> p c f", f=FF),
                    in_=gt[:].rearrange("p (c k f) -> p c f k", k=KF, f=FF),
                    axis=mybir.AxisListType.X, op=mybir.AluOpType.add)
                nc.sync.dma_start(
                    out=outbuf[c0 * P:(c0 + cw) * P, :].rearrange(
                        "(c p) f -> p c f", p=P),
                    in_=S[:])

    nc.finalize()
    return nc


# ----------------------------------------------------------------------------
# PJRT runner (persistent compiled callable, device-resident inputs)
# ----------------------------------------------------------------------------
import numpy as np, time
import jax
import jax.numpy as jnp
from jax.sharding import Mesh, PartitionSpec, NamedSharding
from jax.experimental.shard_map import shard_map
from concourse import mybir
from concourse.bass2jax import _bass_exec_p, partition_id_tensor, install_neuronx_cc_hook


def make_runner(nc, n_cores=8):
    install_neuronx_cc_hook()
    partition_name = nc.partition_id_tensor.name if nc.partition_id_tensor else None
    in_names, out_names, out_avals = [], [], []
    for alloc in nc.m.functions[0].allocations:
        if not isinstance(alloc, mybir.MemoryLocationSet):
            continue
        name = alloc.memorylocations[0].name
        if alloc.kind == "ExternalInput":
            if name != partition_name:
                in_names.append(name)
        elif alloc.kind == "ExternalOutput":
            out_names.append(name)
            out_avals.append(jax.core.ShapedArray(
                tuple(alloc.tensor_shape), mybir.dt.np(alloc.dtype)))
    n_params = len(in_names)
    all_in_names = list(in_names) + list(out_names)
    if partition_name is not None:
        all_in_names.append(partition_name)

    def _body(*args):
        operands = list(args)
        if partition_name is not None:
            operands.append(partition_id_tensor())
        outs = _bass_exec_p.bind(
            *operands,
            out_avals=tuple(out_avals), in_names=tuple(all_in_names),
            out_names=tuple(out_names), lowering_input_output_aliases=(),
            sim_require_finite=False, sim_require_nnan=False, nc=nc)
        return tuple(outs)

    devices = jax.devices()[:n_cores]
    mesh = Mesh(np.asarray(devices), ("core",))
    n_outs = len(out_avals)
    in_specs = (PartitionSpec("core"),) * (n_params + n_outs)
    out_specs = (PartitionSpec("core"),) * len(out_names)
    sharded = jax.jit(shard_map(_body, mesh=mesh, in_specs=in_specs,
                                out_specs=out_specs, check_rep=False),
                      keep_unused=True)
    sharding = NamedSharding(mesh, PartitionSpec("core"))

    state = {}

    def prepare(in_maps):
        per_core = [[np.asarray(m[name]) for name in in_names] for m in in_maps]
        concat_in = [np.concatenate([per_core[c][i] for c in range(n_cores)], axis=0)
                     for i in range(n_params)]
        zeros = [np.zeros((n_cores * av.shape[0], *av.shape[1:]), av.dtype)
                 for av in out_avals]
        state["dev_in"] = [jax.device_put(a, sharding) for a in concat_in + zeros]
        jax.block_until_ready(state["dev_in"])

    def run():
        out = jax.block_until_ready(sharded(*state["dev_in"]))
        return out

    def fetch(out_arrs):
        return [
            {name: np.asarray(out_arrs[i]).reshape(n_cores, *out_avals[i].shape)[c]
             for i, name in enumerate(out_names)}
            for c in range(n_cores)
        ]

    return prepare, run, fetch



_CACHE = {}


def kernel(**inputs):
    for b in ("b1", "b2", "b3", "b4"):
        if b in inputs:
            assert not np.asarray(inputs[b]).any()
    meta, in_maps, reassemble = prep(inputs)
    if "k" not in _CACHE:
        nc = build_kernel(meta)
        _CACHE["k"] = make_runner(nc)
    prepare, run, fetch = _CACHE["k"]
    prepare(in_maps)
    outs = fetch(run())
    return reassemble(outs).astype(np.float32)
